# revision 1
# baseline (speedup 1.0000x reference)
"""Trainium2 Bass kernel for nn_DirectEncodingModel (gnn_message_passing).
Dependency-class pipelined gather design, data-parallel over 8 cores.

Key changes vs v1:
  - Slots of each layer sorted by (source-class, group): x-sourced slots
    gather from xt at t=0 (off the critical path); h_c-sourced slots gather
    right after h_c is written. Critical path only carries the small
    h-class gathers (split over 2 SWDGE queues each).
  - Tight 128-slot blocks (groups may split across blocks; PSUM has_written
    accumulation merges partial group sums). 8 PSUM tiles of 128 outputs
    per layer -> full-width ACT tanh.
  - accT holds only h rows [3072, BS]; x rows always gather from xt input.
"""

import numpy as np
import ml_dtypes

B = 16384
IN = 512
G, F, O = 64, 24, 16
GO_G, GO_F, GO_O = 4, 48, 16
N_CORES = 8
BS = B // N_CORES
HROWS = 3 * G * O  # 3072
NCHUNK = BS // 512

_cache = {}


def _cls_of(v):
    return 0 if v < IN else 1 + (v - IN) // (G * O)


class MM:
    __slots__ = ("blk", "tile", "c0", "c1", "start", "w_off", "w", "stop")

    def __init__(self, blk, tile, c0, c1, start, w_off, w):
        self.blk, self.tile, self.c0, self.c1 = blk, tile, c0, c1
        self.start, self.w_off, self.w = start, w_off, w
        self.stop = False


def build_plan(idx1, idx2, idx3, idxo, W1, W2, W3, Wo, b1, b2, b3, bo):
    """Returns dict with packed arrays + per-layer block/mm/gather structure.

    Layer entries: dict(nblk, idx[int16 nblk*128], segs=[(cls, b0, b1, nidx)],
    mms=[MM...], n_groups, gpt (groups per psum tile)).
    """
    wcols = []          # growing list of stationary columns [128, w]
    bias_cols = []      # [128] per (layer, ptile)
    layers = []

    def pack_layer(idx, W, b, n_g, fan, o):
        gpt = 128 // o  # groups per psum tile (8 hidden, 8 out? o=16 -> 8)
        slots = []
        for g in range(n_g):
            for f in range(fan):
                v = int(idx[g, f])
                slots.append((_cls_of(v), g, v, W[g, f, :]))
        slots.sort(key=lambda s: (s[0], s[1]))
        # segment by class, pad each to 128
        segs_raw = {}
        for s in slots:
            segs_raw.setdefault(s[0], []).append(s)
        padded = []
        segs = []
        for c in sorted(segs_raw):
            lst = segs_raw[c]
            b0 = len(padded) // 128
            while len(lst) % 128:
                lst.append((c, None, 0 if c == 0 else IN, np.zeros(o)))
            padded.extend(lst)
            b1 = len(padded) // 128
            segs.append((c, b0, b1, (b1 - b0) * 128))
        nblk = len(padded) // 128
        idx_arr = np.zeros(nblk * 128, np.int16)
        for i, s in enumerate(padded):
            v = s[2]
            idx_arr[i] = v if s[0] == 0 else v - IN
        # matmul segments per block: one mm per (block, psum tile touched),
        # col range 32-aligned. The first mm of each psum tile covers the
        # full used width with start=True (clears stale has_written);
        # everything else start=False (has_written-accumulate).
        mms = []
        used_w = min(gpt, n_g) * o  # used psum cols (128 hidden, 64 out)
        tile_first_pending = set(range((n_g + gpt - 1) // gpt))
        for blk in range(nblk):
            bslots = padded[blk * 128:(blk + 1) * 128]
            groups = sorted({s[1] for s in bslots if s[1] is not None})
            if not groups:
                continue
            tiles = sorted({g // gpt for g in groups})
            for t in tiles:
                tg = [g for g in groups if g // gpt == t]
                ga, gb = tg[0], tg[-1]
                first = t in tile_first_pending
                if first:
                    tile_first_pending.discard(t)
                    c0, c1 = 0, used_w
                else:
                    c0 = ((ga % gpt) * o) // 32 * 32
                    c1 = -((-((gb % gpt + 1) * o)) // 32) * 32
                    c1 = min(c1, used_w)
                # decompose into buddy-aligned [b0, b0+bw) pieces
                # (bw in {32,64,128}, b0 % bw == 0) per PE tile_position rules
                pieces = []
                p0 = c0
                while p0 < c1:
                    for bw in (128, 64, 32):
                        if p0 % bw == 0 and p0 + bw <= ((c1 + 31) // 32) * 32:
                            pieces.append((p0, min(p0 + bw, c1)))
                            p0 = p0 + bw
                            break
                for (p0c, p1c) in pieces:
                    w = p1c - p0c
                    stat = np.zeros((128, w), np.float32)
                    for r, s in enumerate(bslots):
                        if s[1] is not None and s[1] // gpt == t:
                            cc = (s[1] % gpt) * o - p0c
                            if 0 <= cc <= w - o:
                                stat[r, cc:cc + o] = s[3]
                    w_off = sum(wc.shape[1] for wc in wcols)
                    wcols.append(stat)
                    mms.append(MM(blk, t, p0c, p1c, first, w_off, w))
        last_per_tile = {}
        for i, m in enumerate(mms):
            last_per_tile[m.tile] = i
        for i in last_per_tile.values():
            mms[i].stop = True
        # bias per psum tile
        ntile = (n_g + gpt - 1) // gpt
        bt0 = len(bias_cols)
        for t in range(ntile):
            col = np.zeros(128, np.float32)
            for gi in range(min(gpt, n_g - t * gpt)):
                col[gi * o:(gi + 1) * o] = b[t * gpt + gi, :]
            bias_cols.append(col)
        return dict(nblk=nblk, idx=idx_arr, segs=segs, mms=mms,
                    ntile=ntile, gpt=gpt, o=o, bt0=bt0)

    for idx, W, b in ((idx1, W1, b1), (idx2, W2, b2), (idx3, W3, b3)):
        layers.append(pack_layer(np.asarray(idx), np.asarray(W),
                                 np.asarray(b), G, F, O))
    lo = pack_layer(np.asarray(idxo), np.asarray(Wo), np.asarray(bo),
                    GO_G, GO_F, GO_O)
    # merge x+h1 segments of the out layer (both ready after h1)
    wh = np.concatenate(wcols, axis=1)
    bias = np.stack(bias_cols, axis=1)  # [128, ncols]
    idx_all = np.concatenate([layers[0]["idx"], layers[1]["idx"],
                              layers[2]["idx"], lo["idx"]])
    return dict(layers=layers, lo=lo, wh=wh.astype(ml_dtypes.bfloat16),
                bias=bias.astype(np.float32),
                idx_wrapped=_wrap(idx_all), idx_total=idx_all.shape[0])


def _wrap(idx_list):
    n = idx_list.shape[0]
    w = idx_list.reshape(n // 16, 16).T
    return np.tile(w, (8, 1)).astype(np.int16)


def _apply_tile_patch():
    from concourse import tile as _tile
    from concourse.vector_clock import ScopedClock, VectorClock

    def _patched(self, tick_clock, wait_clock):
        nc = self.nc
        vc = tick_clock.global_clock
        for proc in range(len(vc)):
            tick = vc[proc]
            if tick > 0:
                nop_inst = nc.sync.nop(hint="drain_split_wait", nofuse=True)
                single = VectorClock()
                single.require_at_least(proc, tick)
                wait_clock.add_sem_waits(nop_inst.ins, ScopedClock({None: single}))
        nc.sync.drain()
        nc.all_engine_barrier()
        assert self.sems is not None
        popped = nc._tile_sem_poison_stack.pop()
        assert popped is self._sem_poison
        nc.clear_and_free_semaphores(list(self.sems.allocated().values()))
        nc.all_engine_barrier()

    _tile.TileContext._drain_and_barrier = _patched


def _build_program(plan, with_loop, force_q0=False):
    from concourse import bacc
    import concourse.mybir as mybir
    import concourse.tile as tile
    from concourse.masks import make_identity

    _apply_tile_patch()
    f32, bf16, i16 = mybir.dt.float32, mybir.dt.bfloat16, mybir.dt.int16
    L = plan["layers"]
    lo = plan["lo"]
    wh_cols = plan["wh"].shape[1]
    bias_ncol = plan["bias"].shape[1]
    idx_cols = plan["idx_total"] // 16

    nc = bacc.Bacc(
        "TRN2", target_bir_lowering=False, debug=False, num_devices=N_CORES,
        enable_asserts=False, num_swdge_queues=4,
        dynamic_dma_scratch_size=32768,
    )
    xt_in = nc.dram_tensor("xt", [IN, BS], bf16, kind="ExternalInput")
    idx_in = nc.dram_tensor("idxw", [128, idx_cols], i16, kind="ExternalInput")
    wh_in = nc.dram_tensor("wh", [128, wh_cols], bf16, kind="ExternalInput")
    bias_in = nc.dram_tensor("biasp", [128, bias_ncol], f32, kind="ExternalInput")
    y_out = nc.dram_tensor("y", [BS, 64], f32, kind="ExternalOutput")
    accT = nc.dram_tensor("accT", [HROWS, BS], bf16)
    if with_loop:
        nit_in = nc.dram_tensor("niter", [1, 1], mybir.dt.int32,
                                kind="ExternalInput")

    # idx column offset of each layer
    idx_off = [0]
    for lay in L:
        idx_off.append(idx_off[-1] + lay["nblk"] * 8)
    # queue schedule: (layer, cls) -> list of queues to split across
    qmap = {
        (0, 0): [0, 1, 2, 3], (1, 0): [0, 1], (2, 0): [0],
        (1, 1): [2, 3],
        (2, 1): [1],
        (2, 2): [0, 1],
        (3, 0): [2], (3, 1): [2], (3, 2): [3], (3, 3): [0],
    }

    with tile.TileContext(nc) as tc:
        with (
            tc.tile_pool(name="const", bufs=1) as consts,
            tc.tile_pool(name="ga", bufs=1) as gpa,
            tc.tile_pool(name="gb", bufs=1) as gpb,
            tc.tile_pool(name="hst", bufs=9) as hpool,
            tc.tile_pool(name="ps", bufs=8, space="PSUM") as pspool,
            tc.tile_pool(name="fin", bufs=1) as fpool,
        ):
            idx_t = consts.tile([128, idx_cols], i16)
            nc.sync.dma_start(out=idx_t[:], in_=idx_in[:])
            wh_t = consts.tile([128, wh_cols], bf16)
            nc.sync.dma_start(out=wh_t[:], in_=wh_in[:])
            bias_t = consts.tile([128, bias_ncol], f32)
            nc.sync.dma_start(out=bias_t[:], in_=bias_in[:])
            ident = consts.tile([128, 128], f32)
            make_identity(nc, ident)

            writes = {}  # h-layer (1..3) -> list of dma insts

            def emit_gathers(li, lay, gtile, classes):
                for (c, b0, b1, nidx) in lay["segs"]:
                    if c not in classes:
                        continue
                    qs = qmap.get((li, c), [c % 4])
                    nq = len(qs)
                    # split blocks across queues
                    tb = b1 - b0
                    per = (tb + nq - 1) // nq
                    for j, q in enumerate(qs):
                        sb = b0 + j * per
                        eb = min(b0 + (j + 1) * per, b1)
                        if sb >= eb:
                            continue
                        cnt = (eb - sb) * 128
                        c0 = idx_off[li] + sb * 8
                        gi = nc.gpsimd.dma_gather(
                            out_ap=gtile[:, sb:eb, :],
                            in_ap=(xt_in[:] if c == 0 else accT[:]),
                            idxs_ap=idx_t[:, c0:c0 + cnt // 16],
                            num_idxs=cnt,
                            num_idxs_reg=cnt,
                            elem_size=BS,
                            elem_step=BS,
                            single_packet=False,
                            queue_num=0 if force_q0 else q,
                        )
                        if c > 0:
                            for w in writes.get(c, []):
                                tile.add_dep_helper(gi.ins, w.ins, sync=True)

            def emit_compute(li, lay, gtile, hidden):
                """Returns list of h-write DMA insts (hidden) or None."""
                o = lay["o"]
                wl = []
                hstages = {}
                for ch in range(NCHUNK):
                    ps_tiles = []
                    for t in range(lay["ntile"]):
                        ps = pspool.tile([128, 512], f32, name="ps")
                        ps_tiles.append(ps)
                    prev_inst = {}
                    for mm in lay["mms"]:
                        mi = nc.tensor.matmul(
                            out=ps_tiles[mm.tile][mm.c0:mm.c1, :],
                            lhsT=wh_t[:, mm.w_off:mm.w_off + mm.w],
                            rhs=gtile[:, mm.blk, ch * 512:(ch + 1) * 512],
                            start=mm.start,
                            stop=mm.stop,
                            skip_group_check=True,
                            tile_position=(0, mm.c0),
                        )
                        if mm.tile in prev_inst:
                            tile.add_dep_helper(
                                mi.ins, prev_inst[mm.tile].ins, sync=False)
                        prev_inst[mm.tile] = mi
                    for t in range(lay["ntile"]):
                        m = min(lay["gpt"], (G if hidden else GO_G) - t * lay["gpt"]) * o
                        bcol = lay["bt0"] + t
                        if hidden:
                            if ch == 0:
                                hstages[t] = hpool.tile([128, BS], bf16,
                                                        name="hs")
                            hs = hstages[t]
                            nc.scalar.activation(
                                out=hs[0:m, ch * 512:(ch + 1) * 512],
                                in_=ps_tiles[t][0:m, :],
                                func=mybir.ActivationFunctionType.Tanh,
                                bias=bias_t[0:m, bcol:bcol + 1],
                                scale=1.0,
                            )
                            if ch == NCHUNK - 1:
                                r0 = li * G * O + t * 128
                                eng = nc.sync if t % 2 == 0 else nc.scalar
                                wr = eng.dma_start(
                                    out=accT[r0:r0 + m, :], in_=hs[0:m, :])
                                wl.append(wr)
                        else:
                            nc.vector.tensor_scalar_add(
                                out=yT[0:m, ch * 512:(ch + 1) * 512],
                                in0=ps_tiles[t][0:m, :],
                                scalar1=bias_t[0:m, bcol:bcol + 1],
                            )
                return wl

            def body(iv=None):
                writes.clear()
                gt = {}
                # t=0 gathers: L1 full (x-class) + L2 x-class
                gt[0] = gpa.tile([128, L[0]["nblk"], BS], bf16, name="gA")
                gt[1] = gpb.tile([128, L[1]["nblk"], BS], bf16, name="gB")
                emit_gathers(0, L[0], gt[0], {0})
                emit_gathers(1, L[1], gt[1], {0})

                global yT
                for li in range(3):
                    lay = L[li]
                    writes[li + 1] = emit_compute(li, lay, gt[li], hidden=True)
                    # prefetch notes: after emitting layer li's compute,
                    # emit gathers that depend on h_{li+1} or are prefetch
                    if li == 0:
                        # h1 ready classes: L2h1, L3x, L3h1
                        gt[2] = gpa.tile([128, L[2]["nblk"], BS], bf16,
                                         name="gA")
                        emit_gathers(1, L[1], gt[1], {1})
                        emit_gathers(2, L[2], gt[2], {0, 1})
                    elif li == 1:
                        gt[3] = gpb.tile([128, lo["nblk"], BS], bf16,
                                         name="gB")
                        emit_gathers(2, L[2], gt[2], {2})
                        emit_gathers(3, lo, gt[3], {0, 1, 2})
                    elif li == 2:
                        emit_gathers(3, lo, gt[3], {3})

                yT = fpool.tile([128, BS], f32, name="yT")
                emit_compute(3, lo, gt[3], hidden=False)
                ystage = fpool.tile([128, BS // 128, 64], f32, name="ystage")
                for c in range(BS // 128):
                    pst = pspool.tile([128, 512], f32, name="ps")
                    nc.tensor.transpose(
                        out=pst[0:128, 0:64],
                        in_=yT[0:64, c * 128:(c + 1) * 128],
                        identity=ident[0:64, 0:64],
                    )
                    nc.vector.tensor_copy(out=ystage[:, c, :],
                                          in_=pst[0:128, 0:64])
                nc.sync.dma_start(
                    out=y_out[:].rearrange("(c p) o -> p c o", p=128),
                    in_=ystage[:],
                )

            if with_loop:
                nit_t = consts.tile([1, 1], mybir.dt.int32)
                nc.sync.dma_start(out=nit_t[:], in_=nit_in[:])
                n = nc.values_load(nit_t[0:1, 0:1], min_val=0, max_val=2048,
                                   skip_runtime_bounds_check=True)
                with tc.For_i(0, n, 1):
                    body()
            else:
                body()

    nc.compile()
    return nc


class _Runner:
    """Persistent jitted SPMD executable (adapted from bass2jax)."""

    def __init__(self, nc):
        import jax
        import concourse.mybir as mybir
        from jax.sharding import Mesh, PartitionSpec
        from jax.experimental.shard_map import shard_map
        from concourse.bass2jax import (
            _bass_exec_p, partition_id_tensor, install_neuronx_cc_hook,
        )

        install_neuronx_cc_hook()
        self.jax = jax
        in_names, out_names, out_avals, zero_outs = [], [], [], []
        partition_name = (
            nc.partition_id_tensor.name if nc.partition_id_tensor else None
        )
        for alloc in nc.m.functions[0].allocations:
            if not isinstance(alloc, mybir.MemoryLocationSet):
                continue
            name = alloc.memorylocations[0].name
            if alloc.kind == "ExternalInput":
                if name != partition_name:
                    in_names.append(name)
            elif alloc.kind == "ExternalOutput":
                out_names.append(name)
                shape = tuple(alloc.tensor_shape)
                dtype = mybir.dt.np(alloc.dtype)
                out_avals.append(jax.core.ShapedArray(shape, dtype))
                zero_outs.append(np.zeros(shape, dtype))
        self.n_params = len(in_names)
        self.in_names = in_names[:]
        self.out_names = out_names
        self.out_avals = out_avals
        self.zero_outs = zero_outs
        all_in = in_names + out_names + ([partition_name] if partition_name else [])
        donate = tuple(range(self.n_params, self.n_params + len(out_names)))

        def _body(*args):
            operands = list(args)
            if partition_name is not None:
                operands.append(partition_id_tensor())
            return tuple(
                _bass_exec_p.bind(
                    *operands,
                    out_avals=tuple(out_avals),
                    in_names=tuple(all_in),
                    out_names=tuple(out_names),
                    lowering_input_output_aliases=(),
                    sim_require_finite=True,
                    sim_require_nnan=True,
                    nc=nc,
                )
            )

        devices = jax.devices()[:N_CORES]
        self.mesh = Mesh(np.asarray(devices), ("core",))
        self.sharded = jax.jit(
            shard_map(
                _body, mesh=self.mesh,
                in_specs=(PartitionSpec("core"),) * (self.n_params + len(out_names)),
                out_specs=(PartitionSpec("core"),) * len(out_names),
                check_rep=False,
            ),
            donate_argnums=donate,
            keep_unused=True,
        )

    def prep(self, in_maps, device_put=True):
        per_core = [[np.asarray(m[name]) for name in self.in_names] for m in in_maps]
        arrs = [
            np.concatenate([per_core[c][i] for c in range(N_CORES)], axis=0)
            for i in range(self.n_params)
        ]
        if device_put:
            from jax.sharding import NamedSharding, PartitionSpec

            sh = NamedSharding(self.mesh, PartitionSpec("core"))
            arrs = [self.jax.device_put(a, sh) for a in arrs]
            self.jax.block_until_ready(arrs)
        return arrs

    def run(self, concat_in):
        zeros = [
            np.zeros((N_CORES * z.shape[0], *z.shape[1:]), z.dtype)
            for z in self.zero_outs
        ]
        outs = self.sharded(*concat_in, *zeros)
        self.jax.block_until_ready(outs)
        return outs

    def split(self, out_arrs):
        return [
            {
                name: np.asarray(out_arrs[i]).reshape(
                    N_CORES, *self.out_avals[i].shape
                )[c]
                for i, name in enumerate(self.out_names)
            }
            for c in range(N_CORES)
        ]




def _get(plan_key, plan, with_loop):
    key = (plan_key, with_loop)
    if key not in _cache:
        nc = _build_program(plan, with_loop)
        _cache[key] = _Runner(nc)
    return _cache[key]


def _in_maps(plan, x, niter):
    bf = ml_dtypes.bfloat16
    x = np.asarray(x)
    maps = []
    for c in range(N_CORES):
        xs = x[c * BS:(c + 1) * BS, :]
        m = {
            "xt": np.ascontiguousarray(xs.T).astype(bf),
            "idxw": plan["idx_wrapped"],
            "wh": plan["wh"],
            "biasp": plan["bias"],
        }
        if niter is not None:
            m["niter"] = np.array([[niter]], np.int32)
        maps.append(m)
    return maps


def kernel(**inputs):
    niter = inputs.pop("_niter", None)
    x = inputs.pop("x")
    plan = build_plan(**{k: inputs[k] for k in (
        "idx1", "idx2", "idx3", "idxo", "W1", "W2", "W3", "Wo",
        "b1", "b2", "b3", "bo")})
    r = _get("p0", plan, niter is not None)
    ci = r.prep(_in_maps(plan, x, niter), device_put=False)
    outs = r.split(r.run(ci))
    return np.concatenate(
        [outs[c]["y"] for c in range(N_CORES)], axis=0).astype(np.float32)


def bench(inputs, k_hi=129, rounds=8, per=4):
    """On-device time per kernel-body iteration, measured as the median over
    interleaved A/B rounds of (wall(k_hi) - wall(1)) / (k_hi - 1)."""
    import time

    inputs = dict(inputs)
    x = inputs.pop("x")
    plan = build_plan(**{k: inputs[k] for k in (
        "idx1", "idx2", "idx3", "idxo", "W1", "W2", "W3", "Wo",
        "b1", "b2", "b3", "bo")})
    r = _get("p0", plan, True)
    ci1 = r.prep(_in_maps(plan, x, 1), device_put=True)
    cih = r.prep(_in_maps(plan, x, k_hi), device_put=True)
    outs = r.split(r.run(ci1))
    y1 = np.concatenate(
        [outs[c]["y"] for c in range(N_CORES)], axis=0).astype(np.float32)
    outs = r.split(r.run(cih))
    yh = np.concatenate(
        [outs[c]["y"] for c in range(N_CORES)], axis=0).astype(np.float32)
    diffs = []
    for _ in range(rounds):
        t1s, ths = [], []
        for _ in range(per):
            t0 = time.perf_counter(); r.run(ci1)
            t1s.append(time.perf_counter() - t0)
            t0 = time.perf_counter(); r.run(cih)
            ths.append(time.perf_counter() - t0)
        diffs.append((min(ths) - min(t1s)) / (k_hi - 1))
    diffs.sort()
    return diffs[len(diffs) // 2], y1, yh



# revision 5
# speedup vs baseline: 1.1762x; 1.1762x over previous
"""Trainium2 Bass kernel for nn_DirectEncodingModel (gnn_message_passing).
Dependency-class pipelined gather design, data-parallel over 8 cores.

v3 changes vs baseline:
  - Batch split in 2 halves (1024 cols each); gather pools double-buffered
    so halves and loop iterations pipeline (fills the DMA idle gaps during
    L2/L3 compute and overlaps next-iter L1 gathers with this-iter tail).
  - Output layer's h3-sourced slots computed via merged "scatter-weight"
    matmuls directly from the SBUF-resident h3 tiles: h3 is never written
    to HBM and never gathered back (removes the full-h3-write -> tiny-
    gather serial tail).
  - accT shrinks to h1+h2 (2048 rows); class index mapping v-512.
  - Class pad slots use idx=-1 (trailing negatives move no bytes); gather
    pool buffers are memset once pre-loop so pad lanes stay finite.
  - y emitted transposed [2, 64, 1024] and untransposed on host (drops the
    PE transposes + strided y DMA).
"""

import numpy as np
import ml_dtypes

B = 16384
IN = 512
G, F, O = 64, 24, 16
GO_G, GO_F, GO_O = 4, 48, 16
N_CORES = 8
BS = B // N_CORES          # 2048 batch per core
HB = BS // 2               # 1024 per half
NCH = HB // 512            # 2 chunks per half
HROWS = 2 * G * O          # 2048 rows in accT (h1+h2)
H3_BASE = IN + 2 * G * O   # 2560: global col where h3 starts

_cache = {}


def _cls_of(v):
    return 0 if v < IN else 1 + (v - IN) // (G * O)


class MM:
    __slots__ = ("blk", "tile", "c0", "c1", "start", "w_off", "w", "stop")

    def __init__(self, blk, tile, c0, c1, start, w_off, w):
        self.blk, self.tile, self.c0, self.c1 = blk, tile, c0, c1
        self.start, self.w_off, self.w = start, w_off, w
        self.stop = False


def build_plan(idx1, idx2, idx3, idxo, W1, W2, W3, Wo, b1, b2, b3, bo):
    """Packed stationary weights + per-layer block/mm/gather structure."""
    wcols = []          # stationary column groups [128, w]
    bias_cols = []      # [128] per (layer, ptile)
    layers = []

    def pack_layer(idx, W, b, n_g, fan, o, scatter_cls=()):
        gpt = 128 // o  # groups per psum tile
        slots = []
        scat = []
        for g in range(n_g):
            for f in range(fan):
                v = int(idx[g, f])
                c = _cls_of(v)
                if c in scatter_cls:
                    scat.append((g, v, W[g, f, :]))
                else:
                    slots.append((c, g, v, W[g, f, :]))
        slots.sort(key=lambda s: (s[0], s[1]))
        segs_raw = {}
        for s in slots:
            segs_raw.setdefault(s[0], []).append(s)
        padded = []
        segs = []
        for c in sorted(segs_raw):
            lst = segs_raw[c]
            b0 = len(padded) // 128
            npad = (-len(lst)) % 128
            lst = lst + [(c, None, -1, np.zeros(o))] * npad
            padded.extend(lst)
            b1_ = len(padded) // 128
            segs.append((c, b0, b1_, (b1_ - b0) * 128))
        nblk = len(padded) // 128
        idx_arr = np.zeros(nblk * 128, np.int16)
        for i, s in enumerate(padded):
            v = s[2]
            idx_arr[i] = -1 if s[1] is None else (v if s[0] == 0 else v - IN)
        used_w = min(gpt, n_g) * o
        mms = []
        tile_first_pending = set(range((n_g + gpt - 1) // gpt))
        for blk in range(nblk):
            bslots = padded[blk * 128:(blk + 1) * 128]
            groups = sorted({s[1] for s in bslots if s[1] is not None})
            if not groups:
                continue
            tiles = sorted({g // gpt for g in groups})
            for t in tiles:
                tg = [g for g in groups if g // gpt == t]
                ga, gb = tg[0], tg[-1]
                first = t in tile_first_pending
                if first:
                    tile_first_pending.discard(t)
                    c0, c1 = 0, used_w
                else:
                    c0 = ((ga % gpt) * o) // 32 * 32
                    c1 = -((-((gb % gpt + 1) * o)) // 32) * 32
                    c1 = min(c1, used_w)
                pieces = []
                p0 = c0
                while p0 < c1:
                    for bw in (128, 64, 32):
                        if p0 % bw == 0 and p0 + bw <= ((c1 + 31) // 32) * 32:
                            pieces.append((p0, min(p0 + bw, c1)))
                            p0 = p0 + bw
                            break
                for (p0c, p1c) in pieces:
                    w = p1c - p0c
                    stat = np.zeros((128, w), np.float32)
                    for r, s in enumerate(bslots):
                        if s[1] is not None and s[1] // gpt == t:
                            cc = (s[1] % gpt) * o - p0c
                            if 0 <= cc <= w - o:
                                stat[r, cc:cc + o] = s[3]
                    w_off = sum(wc.shape[1] for wc in wcols)
                    wcols.append(stat)
                    mms.append(MM(blk, t, p0c, p1c, first, w_off, w))
        # scatter-weight matmuls: one per 128-row h3 source tile that is
        # actually referenced; stationary [128 h3-rows, used_w] merges all
        # scattered slots hitting that tile.
        scat_mms = []  # (src_tile, w_off, w)
        if scat:
            by_tile = {}
            for (g, v, wrow) in scat:
                r = v - H3_BASE
                by_tile.setdefault(r // 128, []).append((r % 128, g, wrow))
            for st in sorted(by_tile):
                stat = np.zeros((128, used_w), np.float32)
                for (p, g, wrow) in by_tile[st]:
                    stat[p, g * o:(g + 1) * o] += wrow
                w_off = sum(wc.shape[1] for wc in wcols)
                wcols.append(stat)
                scat_mms.append((st, w_off, used_w))
        if mms:
            last = {}
            for i, m in enumerate(mms):
                last[m.tile] = i
            for t, i in last.items():
                if not (t == 0 and scat_mms):
                    mms[i].stop = True
        ntile = (n_g + gpt - 1) // gpt
        bt0 = len(bias_cols)
        for t in range(ntile):
            col = np.zeros(128, np.float32)
            for gi in range(min(gpt, n_g - t * gpt)):
                col[gi * o:(gi + 1) * o] = b[t * gpt + gi, :]
            bias_cols.append(col)
        return dict(nblk=nblk, idx=idx_arr, segs=segs, mms=mms,
                    scat_mms=scat_mms, ntile=ntile, gpt=gpt, o=o, bt0=bt0)

    for idx, W, b in ((idx1, W1, b1), (idx2, W2, b2), (idx3, W3, b3)):
        layers.append(pack_layer(np.asarray(idx), np.asarray(W),
                                 np.asarray(b), G, F, O))
    lo = pack_layer(np.asarray(idxo), np.asarray(Wo), np.asarray(bo),
                    GO_G, GO_F, GO_O, scatter_cls=(3,))
    wh = np.concatenate(wcols, axis=1)
    bias = np.stack(bias_cols, axis=1)  # [128, ncols]
    idx_all = np.concatenate([layers[0]["idx"], layers[1]["idx"],
                              layers[2]["idx"], lo["idx"]])
    return dict(layers=layers, lo=lo, wh=wh.astype(ml_dtypes.bfloat16),
                bias=bias.astype(np.float32),
                idx_wrapped=_wrap(idx_all), idx_total=idx_all.shape[0])


def _wrap(idx_list):
    n = idx_list.shape[0]
    w = idx_list.reshape(n // 16, 16).T
    return np.tile(w, (8, 1)).astype(np.int16)


def _apply_tile_patch():
    from concourse import tile as _tile
    from concourse.vector_clock import ScopedClock, VectorClock

    def _patched(self, tick_clock, wait_clock):
        nc = self.nc
        vc = tick_clock.global_clock
        for proc in range(len(vc)):
            tick = vc[proc]
            if tick > 0:
                nop_inst = nc.sync.nop(hint="drain_split_wait", nofuse=True)
                single = VectorClock()
                single.require_at_least(proc, tick)
                wait_clock.add_sem_waits(nop_inst.ins, ScopedClock({None: single}))
        nc.sync.drain()
        nc.all_engine_barrier()
        assert self.sems is not None
        popped = nc._tile_sem_poison_stack.pop()
        assert popped is self._sem_poison
        nc.clear_and_free_semaphores(list(self.sems.allocated().values()))
        nc.all_engine_barrier()

    _tile.TileContext._drain_and_barrier = _patched


def _build_program(plan, with_loop, nbody=1):
    from concourse import bacc
    import concourse.mybir as mybir
    import concourse.tile as tile

    _apply_tile_patch()
    f32, bf16, i16 = mybir.dt.float32, mybir.dt.bfloat16, mybir.dt.int16
    L = plan["layers"]
    lo = plan["lo"]
    wh_cols = plan["wh"].shape[1]
    bias_ncol = plan["bias"].shape[1]
    idx_cols = plan["idx_total"] // 16

    nc = bacc.Bacc(
        "TRN2", target_bir_lowering=False, debug=False, num_devices=N_CORES,
        enable_asserts=False, num_swdge_queues=4,
        dynamic_dma_scratch_size=32768,
    )
    xt_in = nc.dram_tensor("xt", [IN, BS], bf16, kind="ExternalInput")
    idx_in = nc.dram_tensor("idxw", [128, idx_cols], i16, kind="ExternalInput")
    wh_in = nc.dram_tensor("wh", [128, wh_cols], bf16, kind="ExternalInput")
    bias_in = nc.dram_tensor("biasp", [128, bias_ncol], f32, kind="ExternalInput")
    y_out = nc.dram_tensor("y", [2, GO_G * GO_O, HB], f32, kind="ExternalOutput")
    accT = nc.dram_tensor("accT", [HROWS, BS], bf16)
    if with_loop:
        nit_in = nc.dram_tensor("niter", [1, 1], mybir.dt.int32,
                                kind="ExternalInput")

    idx_off = [0]
    for lay in L:
        idx_off.append(idx_off[-1] + lay["nblk"] * 8)
    # queues: per-half lanes. q0/q1 = x-class H0/H1, q2/q3 = h-class H0/H1.
    nblkA = max(L[0]["nblk"], L[2]["nblk"])
    nblkB = max(L[1]["nblk"], lo["nblk"])

    with tile.TileContext(nc) as tc:
        with (
            tc.tile_pool(name="const", bufs=1) as consts,
            tc.tile_pool(name="ga", bufs=2) as gpa,
            tc.tile_pool(name="gb", bufs=2) as gpb,
            tc.tile_pool(name="hst", bufs=16) as hpool,
            tc.tile_pool(name="ps", bufs=8, space="PSUM") as pspool,
            tc.tile_pool(name="fin", bufs=2) as fpool,
        ):
            idx_t = consts.tile([128, idx_cols], i16)
            nc.sync.dma_start(out=idx_t[:], in_=idx_in[:])
            wh_t = consts.tile([128, wh_cols], bf16)
            nc.sync.dma_start(out=wh_t[:], in_=wh_in[:])
            bias_t = consts.tile([128, bias_ncol], f32)
            nc.sync.dma_start(out=bias_t[:], in_=bias_in[:])

            # zero the gather pool buffers once so pad lanes stay finite
            for pool, nb, nm in ((gpa, nblkA, "gA"), (gpb, nblkB, "gB")):
                for _ in range(2):
                    z = pool.tile([128, nb, HB], bf16, name=nm)
                    nc.vector.memset(z[:], 0)

            writes = {}   # (h-layer 1|2, half) -> list of dma insts
            hstages = {}  # (layer, half, tile) -> sbuf tile

            def emit_gathers(li, lay, half, gtile, classes):
                for (c, b0, b1, nidx) in lay["segs"]:
                    if c not in classes:
                        continue
                    q = (0 if c == 0 else 2) + half
                    cnt = (b1 - b0) * 128
                    c0 = idx_off[li] + b0 * 8
                    src = xt_in if c == 0 else accT
                    gi = nc.gpsimd.dma_gather(
                        out_ap=gtile[:, b0:b1, :],
                        in_ap=src[:, half * HB:(half + 1) * HB],
                        idxs_ap=idx_t[:, c0:c0 + cnt // 16],
                        num_idxs=cnt,
                        num_idxs_reg=cnt,
                        elem_size=HB,
                        elem_step=BS,
                        single_packet=False,
                        queue_num=q,
                    )
                    if c > 0:
                        for w in writes.get((c, half), []):
                            tile.add_dep_helper(gi.ins, w.ins, sync=True)

            def emit_compute(li, lay, half, gtile, hidden):
                o = lay["o"]
                wl = []
                for ch in range(NCH):
                    ps_tiles = []
                    for t in range(lay["ntile"]):
                        ps = pspool.tile([128, 512], f32, name="ps")
                        ps_tiles.append(ps)
                    prev_inst = {}
                    for mm in lay["mms"]:
                        mi = nc.tensor.matmul(
                            out=ps_tiles[mm.tile][mm.c0:mm.c1, :],
                            lhsT=wh_t[:, mm.w_off:mm.w_off + mm.w],
                            rhs=gtile[:, mm.blk, ch * 512:(ch + 1) * 512],
                            start=mm.start,
                            stop=mm.stop,
                            skip_group_check=True,
                            tile_position=(0, mm.c0),
                        )
                        if mm.tile in prev_inst:
                            tile.add_dep_helper(
                                mi.ins, prev_inst[mm.tile].ins, sync=False)
                        prev_inst[mm.tile] = mi
                    for si, (st, w_off, w) in enumerate(lay["scat_mms"]):
                        h3 = hstages[(2, half, st)]
                        mi = nc.tensor.matmul(
                            out=ps_tiles[0][0:w, :],
                            lhsT=wh_t[:, w_off:w_off + w],
                            rhs=h3[:, ch * 512:(ch + 1) * 512],
                            start=False,
                            stop=si == len(lay["scat_mms"]) - 1,
                            skip_group_check=True,
                            tile_position=(0, 0),
                        )
                        if 0 in prev_inst:
                            tile.add_dep_helper(
                                mi.ins, prev_inst[0].ins, sync=False)
                        prev_inst[0] = mi
                    for t in range(lay["ntile"]):
                        m = min(lay["gpt"], (G if hidden else GO_G) - t * lay["gpt"]) * o
                        bcol = lay["bt0"] + t
                        if hidden:
                            if ch == 0:
                                hstages[(li, half, t)] = hpool.tile(
                                    [128, HB], bf16, name="hs")
                            hs = hstages[(li, half, t)]
                            nc.scalar.activation(
                                out=hs[0:m, ch * 512:(ch + 1) * 512],
                                in_=ps_tiles[t][0:m, :],
                                func=mybir.ActivationFunctionType.Tanh,
                                bias=bias_t[0:m, bcol:bcol + 1],
                                scale=1.0,
                            )
                            if ch == NCH - 1 and li < 2:
                                r0 = li * G * O + t * 128
                                wr = nc.sync.dma_start(
                                    out=accT[r0:r0 + m,
                                             half * HB:(half + 1) * HB],
                                    in_=hs[0:m, :])
                                wl.append(wr)
                        else:
                            nc.vector.tensor_scalar_add(
                                out=yT[0:m, ch * 512:(ch + 1) * 512],
                                in0=ps_tiles[t][0:m, :],
                                scalar1=bias_t[0:m, bcol:bcol + 1],
                            )
                return wl

            def body(iv=None):
                global yT
                writes.clear()
                gt = {}
                for h in (0, 1):
                    gt[(0, h)] = gpa.tile([128, nblkA, HB], bf16, name="gA")
                    emit_gathers(0, L[0], h, gt[(0, h)], {0})
                for h in (0, 1):
                    gt[(1, h)] = gpb.tile([128, nblkB, HB], bf16, name="gB")
                    emit_gathers(1, L[1], h, gt[(1, h)], {0})
                for h in (0, 1):
                    writes[(1, h)] = emit_compute(0, L[0], h, gt[(0, h)], True)
                    gt[(2, h)] = gpa.tile([128, nblkA, HB], bf16, name="gA")
                    emit_gathers(1, L[1], h, gt[(1, h)], {1})
                    emit_gathers(2, L[2], h, gt[(2, h)], {0, 1})
                for h in (0, 1):
                    writes[(2, h)] = emit_compute(1, L[1], h, gt[(1, h)], True)
                    gt[(3, h)] = gpb.tile([128, nblkB, HB], bf16, name="gB")
                    emit_gathers(2, L[2], h, gt[(2, h)], {2})
                    emit_gathers(3, lo, h, gt[(3, h)], {0, 1})
                for h in (0, 1):
                    emit_compute(2, L[2], h, gt[(2, h)], True)
                    emit_gathers(3, lo, h, gt[(3, h)], {2})
                for h in (0, 1):
                    yT = fpool.tile([GO_G * GO_O, HB], f32, name="yT")
                    emit_compute(3, lo, h, gt[(3, h)], False)
                    nc.sync.dma_start(out=y_out[h], in_=yT[:])

            if with_loop:
                nit_t = consts.tile([1, 1], mybir.dt.int32)
                nc.sync.dma_start(out=nit_t[:], in_=nit_in[:])
                n = nc.values_load(nit_t[0:1, 0:1], min_val=0, max_val=2048,
                                   skip_runtime_bounds_check=True)
                with tc.For_i(0, n, 1):
                    body()
            else:
                body()

    nc.compile()
    return nc


class _Runner:
    """Persistent jitted SPMD executable (adapted from bass2jax)."""

    def __init__(self, nc):
        import jax
        import concourse.mybir as mybir
        from jax.sharding import Mesh, PartitionSpec
        from jax.experimental.shard_map import shard_map
        from concourse.bass2jax import (
            _bass_exec_p, partition_id_tensor, install_neuronx_cc_hook,
        )

        install_neuronx_cc_hook()
        self.jax = jax
        in_names, out_names, out_avals, zero_outs = [], [], [], []
        partition_name = (
            nc.partition_id_tensor.name if nc.partition_id_tensor else None
        )
        for alloc in nc.m.functions[0].allocations:
            if not isinstance(alloc, mybir.MemoryLocationSet):
                continue
            name = alloc.memorylocations[0].name
            if alloc.kind == "ExternalInput":
                if name != partition_name:
                    in_names.append(name)
            elif alloc.kind == "ExternalOutput":
                out_names.append(name)
                shape = tuple(alloc.tensor_shape)
                dtype = mybir.dt.np(alloc.dtype)
                out_avals.append(jax.core.ShapedArray(shape, dtype))
                zero_outs.append(np.zeros(shape, dtype))
        self.n_params = len(in_names)
        self.in_names = in_names[:]
        self.out_names = out_names
        self.out_avals = out_avals
        self.zero_outs = zero_outs
        all_in = in_names + out_names + ([partition_name] if partition_name else [])
        donate = tuple(range(self.n_params, self.n_params + len(out_names)))

        def _body(*args):
            operands = list(args)
            if partition_name is not None:
                operands.append(partition_id_tensor())
            return tuple(
                _bass_exec_p.bind(
                    *operands,
                    out_avals=tuple(out_avals),
                    in_names=tuple(all_in),
                    out_names=tuple(out_names),
                    lowering_input_output_aliases=(),
                    sim_require_finite=True,
                    sim_require_nnan=True,
                    nc=nc,
                )
            )

        devices = jax.devices()[:N_CORES]
        self.mesh = Mesh(np.asarray(devices), ("core",))
        self.sharded = jax.jit(
            shard_map(
                _body, mesh=self.mesh,
                in_specs=(PartitionSpec("core"),) * (self.n_params + len(out_names)),
                out_specs=(PartitionSpec("core"),) * len(out_names),
                check_rep=False,
            ),
            donate_argnums=donate,
            keep_unused=True,
        )

    def prep(self, in_maps, device_put=True):
        per_core = [[np.asarray(m[name]) for name in self.in_names] for m in in_maps]
        arrs = [
            np.concatenate([per_core[c][i] for c in range(N_CORES)], axis=0)
            for i in range(self.n_params)
        ]
        if device_put:
            from jax.sharding import NamedSharding, PartitionSpec

            sh = NamedSharding(self.mesh, PartitionSpec("core"))
            arrs = [self.jax.device_put(a, sh) for a in arrs]
            self.jax.block_until_ready(arrs)
        return arrs

    def run(self, concat_in):
        zeros = [
            np.zeros((N_CORES * z.shape[0], *z.shape[1:]), z.dtype)
            for z in self.zero_outs
        ]
        outs = self.sharded(*concat_in, *zeros)
        self.jax.block_until_ready(outs)
        return outs

    def split(self, out_arrs):
        return [
            {
                name: np.asarray(out_arrs[i]).reshape(
                    N_CORES, *self.out_avals[i].shape
                )[c]
                for i, name in enumerate(self.out_names)
            }
            for c in range(N_CORES)
        ]


def _get(plan_key, plan, with_loop):
    key = (plan_key, with_loop)
    if key not in _cache:
        nc = _build_program(plan, with_loop)
        _cache[key] = _Runner(nc)
    return _cache[key]


def _in_maps(plan, x, niter):
    bf = ml_dtypes.bfloat16
    x = np.asarray(x)
    maps = []
    for c in range(N_CORES):
        xs = x[c * BS:(c + 1) * BS, :]
        m = {
            "xt": np.ascontiguousarray(xs.T).astype(bf),
            "idxw": plan["idx_wrapped"],
            "wh": plan["wh"],
            "biasp": plan["bias"],
        }
        if niter is not None:
            m["niter"] = np.array([[niter]], np.int32)
        maps.append(m)
    return maps


def _post(y_dev):
    """[2, 64, HB] device output -> [BS, 64]."""
    return np.concatenate([y_dev[0].T, y_dev[1].T], axis=0)


def kernel(**inputs):
    niter = inputs.pop("_niter", None)
    x = inputs.pop("x")
    plan = build_plan(**{k: inputs[k] for k in (
        "idx1", "idx2", "idx3", "idxo", "W1", "W2", "W3", "Wo",
        "b1", "b2", "b3", "bo")})
    r = _get("p0", plan, niter is not None)
    ci = r.prep(_in_maps(plan, x, niter), device_put=False)
    outs = r.split(r.run(ci))
    return np.concatenate(
        [_post(outs[c]["y"]) for c in range(N_CORES)], axis=0
    ).astype(np.float32)


def bench(inputs, k_hi=129, rounds=8, per=4):
    """On-device time per kernel-body iteration, measured as the median over
    interleaved A/B rounds of (wall(k_hi) - wall(1)) / (k_hi - 1)."""
    import time

    inputs = dict(inputs)
    x = inputs.pop("x")
    plan = build_plan(**{k: inputs[k] for k in (
        "idx1", "idx2", "idx3", "idxo", "W1", "W2", "W3", "Wo",
        "b1", "b2", "b3", "bo")})
    r = _get("p0", plan, True)
    ci1 = r.prep(_in_maps(plan, x, 1), device_put=True)
    cih = r.prep(_in_maps(plan, x, k_hi), device_put=True)
    outs = r.split(r.run(ci1))
    y1 = np.concatenate(
        [_post(outs[c]["y"]) for c in range(N_CORES)], axis=0).astype(np.float32)
    outs = r.split(r.run(cih))
    yh = np.concatenate(
        [_post(outs[c]["y"]) for c in range(N_CORES)], axis=0).astype(np.float32)
    diffs = []
    for _ in range(rounds):
        t1s, ths = [], []
        for _ in range(per):
            t0 = time.perf_counter(); r.run(ci1)
            t1s.append(time.perf_counter() - t0)
            t0 = time.perf_counter(); r.run(cih)
            ths.append(time.perf_counter() - t0)
        diffs.append((min(ths) - min(t1s)) / (k_hi - 1))
    diffs.sort()
    return diffs[len(diffs) // 2], y1, yh


# revision 25
# speedup vs baseline: 1.3984x; 1.1890x over previous
"""Trainium2 Bass kernel for nn_DirectEncodingModel (gnn_message_passing).
Dependency-class pipelined gather design, data-parallel over 8 cores.

v3 changes vs baseline:
  - Batch split in 2 halves (1024 cols each); gather pools double-buffered
    so halves and loop iterations pipeline (fills the DMA idle gaps during
    L2/L3 compute and overlaps next-iter L1 gathers with this-iter tail).
  - Output layer's h3-sourced slots computed via merged "scatter-weight"
    matmuls directly from the SBUF-resident h3 tiles: h3 is never written
    to HBM and never gathered back (removes the full-h3-write -> tiny-
    gather serial tail).
  - accT shrinks to h1+h2 (2048 rows); class index mapping v-512.
  - Class pad slots use idx=-1 (trailing negatives move no bytes); gather
    pool buffers are memset once pre-loop so pad lanes stay finite.
  - y emitted transposed [2, 64, 1024] and untransposed on host (drops the
    PE transposes + strided y DMA).
"""

import numpy as np
import ml_dtypes

B = 16384
IN = 512
G, F, O = 64, 24, 16
GO_G, GO_F, GO_O = 4, 48, 16
N_CORES = 8
BS = B // N_CORES          # 2048 batch per core
HB = BS // 2               # 1024 per half
NCH = HB // 512            # 2 chunks per half
HROWS = 2 * G * O          # 2048 rows in accT (h1+h2)
H3_BASE = IN + 2 * G * O   # 2560: global col where h3 starts
UNROLL = 4                 # computes per hardware loop iteration

_cache = {}


def _cls_of(v):
    return 0 if v < IN else 1 + (v - IN) // (G * O)


class MM:
    __slots__ = ("blk", "tile", "c0", "c1", "start", "w_off", "w", "stop")

    def __init__(self, blk, tile, c0, c1, start, w_off, w):
        self.blk, self.tile, self.c0, self.c1 = blk, tile, c0, c1
        self.start, self.w_off, self.w = start, w_off, w
        self.stop = False


def build_plan(idx1, idx2, idx3, idxo, W1, W2, W3, Wo, b1, b2, b3, bo):
    """Packed stationary weights + per-layer block/mm/gather structure."""
    wcols = []          # stationary column groups [128, w]
    bias_cols = []      # [128] per (layer, ptile)
    layers = []

    def pack_layer(idx, W, b, n_g, fan, o, scatter_cls=(), merge_h=False):
        gpt = 128 // o  # groups per psum tile
        slots = []
        scat = []
        for g in range(n_g):
            for f in range(fan):
                v = int(idx[g, f])
                c = _cls_of(v)
                if c in scatter_cls:
                    scat.append((g, v, W[g, f, :]))
                else:
                    slots.append((c, g, v, W[g, f, :]))
        # optionally merge all h classes into one segment (same source
        # tensor accT); the gather waits for the latest h-layer referenced.
        ckey = (lambda c: min(c, 1)) if merge_h else (lambda c: c)
        slots.sort(key=lambda s: (ckey(s[0]), s[1]))
        segs_raw = {}
        for s in slots:
            segs_raw.setdefault(ckey(s[0]), []).append(s)
        padded = []
        segs = []  # (key 0|1, maxc, b0, b1)
        for c in sorted(segs_raw):
            lst = segs_raw[c]
            maxc = max(s[0] for s in lst)
            b0 = len(padded) // 128
            npad = (-len(lst)) % 128
            lst = lst + [(c, None, -1, np.zeros(o))] * npad
            padded.extend(lst)
            b1_ = len(padded) // 128
            segs.append((c, maxc, b0, b1_))
        nblk = len(padded) // 128
        idx_arr = np.zeros(nblk * 128, np.int16)
        for i, s in enumerate(padded):
            v = s[2]
            idx_arr[i] = -1 if s[1] is None else (v if s[0] == 0 else v - IN)
        used_w = min(gpt, n_g) * o
        mms = []
        tile_first_pending = set(range((n_g + gpt - 1) // gpt))
        for blk in range(nblk):
            bslots = padded[blk * 128:(blk + 1) * 128]
            groups = sorted({s[1] for s in bslots if s[1] is not None})
            if not groups:
                continue
            tiles = sorted({g // gpt for g in groups})
            for t in tiles:
                tg = [g for g in groups if g // gpt == t]
                ga, gb = tg[0], tg[-1]
                first = t in tile_first_pending
                if first:
                    tile_first_pending.discard(t)
                    c0, c1 = 0, used_w
                else:
                    c0 = ((ga % gpt) * o) // 32 * 32
                    c1 = -((-((gb % gpt + 1) * o)) // 32) * 32
                    c1 = min(c1, used_w)
                pieces = []
                p0 = c0
                while p0 < c1:
                    for bw in (128, 64, 32):
                        if p0 % bw == 0 and p0 + bw <= ((c1 + 31) // 32) * 32:
                            pieces.append((p0, min(p0 + bw, c1)))
                            p0 = p0 + bw
                            break
                for (p0c, p1c) in pieces:
                    w = p1c - p0c
                    stat = np.zeros((128, w), np.float32)
                    for r, s in enumerate(bslots):
                        if s[1] is not None and s[1] // gpt == t:
                            cc = (s[1] % gpt) * o - p0c
                            if 0 <= cc <= w - o:
                                stat[r, cc:cc + o] = s[3]
                    w_off = sum(wc.shape[1] for wc in wcols)
                    wcols.append(stat)
                    mms.append(MM(blk, t, p0c, p1c, first, w_off, w))
        # scatter-weight matmuls: one per 128-row h3 source tile that is
        # actually referenced; stationary [128 h3-rows, used_w] merges all
        # scattered slots hitting that tile.
        scat_mms = []  # (src_tile, w_off, w)
        if scat:
            by_tile = {}
            for (g, v, wrow) in scat:
                r = v - H3_BASE
                by_tile.setdefault(r // 128, []).append((r % 128, g, wrow))
            for st in sorted(by_tile):
                stat = np.zeros((128, used_w), np.float32)
                for (p, g, wrow) in by_tile[st]:
                    stat[p, g * o:(g + 1) * o] += wrow
                w_off = sum(wc.shape[1] for wc in wcols)
                wcols.append(stat)
                scat_mms.append((st, w_off, used_w))
        if mms:
            last = {}
            for i, m in enumerate(mms):
                last[m.tile] = i
            for t, i in last.items():
                if not (t == 0 and scat_mms):
                    mms[i].stop = True
        ntile = (n_g + gpt - 1) // gpt
        bt0 = len(bias_cols)
        for t in range(ntile):
            col = np.zeros(128, np.float32)
            for gi in range(min(gpt, n_g - t * gpt)):
                col[gi * o:(gi + 1) * o] = b[t * gpt + gi, :]
            bias_cols.append(col)
        return dict(nblk=nblk, idx=idx_arr, segs=segs, mms=mms,
                    scat_mms=scat_mms, ntile=ntile, gpt=gpt, o=o, bt0=bt0)

    for idx, W, b in ((idx1, W1, b1), (idx2, W2, b2), (idx3, W3, b3)):
        layers.append(pack_layer(np.asarray(idx), np.asarray(W),
                                 np.asarray(b), G, F, O))
    lo = pack_layer(np.asarray(idxo), np.asarray(Wo), np.asarray(bo),
                    GO_G, GO_F, GO_O, scatter_cls=(3,), merge_h=True)
    wh = np.concatenate(wcols, axis=1)
    bias = np.stack(bias_cols, axis=1)  # [128, ncols]
    idx_all = np.concatenate([layers[0]["idx"], layers[1]["idx"],
                              layers[2]["idx"], lo["idx"]])
    return dict(layers=layers, lo=lo, wh=wh.astype(ml_dtypes.bfloat16),
                bias=bias.astype(np.float32),
                idx_wrapped=_wrap(idx_all), idx_total=idx_all.shape[0])


def _wrap(idx_list):
    n = idx_list.shape[0]
    w = idx_list.reshape(n // 16, 16).T
    return np.tile(w, (8, 1)).astype(np.int16)


def _apply_tile_patch():
    from concourse import tile as _tile
    from concourse.vector_clock import ScopedClock, VectorClock

    def _patched(self, tick_clock, wait_clock):
        nc = self.nc
        vc = tick_clock.global_clock
        for proc in range(len(vc)):
            tick = vc[proc]
            if tick > 0:
                nop_inst = nc.sync.nop(hint="drain_split_wait", nofuse=True)
                single = VectorClock()
                single.require_at_least(proc, tick)
                wait_clock.add_sem_waits(nop_inst.ins, ScopedClock({None: single}))
        nc.sync.drain()
        nc.all_engine_barrier()
        assert self.sems is not None
        popped = nc._tile_sem_poison_stack.pop()
        assert popped is self._sem_poison
        nc.clear_and_free_semaphores(list(self.sems.allocated().values()))
        nc.all_engine_barrier()

    _tile.TileContext._drain_and_barrier = _patched


def _build_program(plan, with_loop, nbody=1, mode="full"):
    from concourse import bacc
    import concourse.mybir as mybir
    import concourse.tile as tile

    _apply_tile_patch()
    f32, bf16, i16 = mybir.dt.float32, mybir.dt.bfloat16, mybir.dt.int16
    L = plan["layers"]
    lo = plan["lo"]
    wh_cols = plan["wh"].shape[1]
    bias_ncol = plan["bias"].shape[1]
    idx_cols = plan["idx_total"] // 16

    nc = bacc.Bacc(
        "TRN2", target_bir_lowering=False, debug=False, num_devices=N_CORES,
        enable_asserts=False, num_swdge_queues=4,
        dynamic_dma_scratch_size=32768,
    )
    xt_in = nc.dram_tensor("xt", [IN, BS], bf16, kind="ExternalInput")
    idx_in = nc.dram_tensor("idxw", [128, idx_cols], i16, kind="ExternalInput")
    wh_in = nc.dram_tensor("wh", [128, wh_cols], bf16, kind="ExternalInput")
    bias_in = nc.dram_tensor("biasp", [128, bias_ncol], f32, kind="ExternalInput")
    y_out = nc.dram_tensor("y", [2, GO_G * GO_O, HB], f32, kind="ExternalOutput")
    accT = nc.dram_tensor("accT", [HROWS, BS], bf16)
    if with_loop:
        nit_in = nc.dram_tensor("niter", [1, 1], mybir.dt.int32,
                                kind="ExternalInput")

    idx_off = [0]
    for lay in L:
        idx_off.append(idx_off[-1] + lay["nblk"] * 8)
    # queues: per-half lanes. q0/q1 = x-class H0/H1, q2/q3 = h-class H0/H1.
    nblkA = max(L[0]["nblk"], L[2]["nblk"])
    nblkB = max(L[1]["nblk"], lo["nblk"])

    with tile.TileContext(nc) as tc:
        with (
            tc.tile_pool(name="const", bufs=1) as consts,
            tc.tile_pool(name="ga", bufs=2) as gpa,
            tc.tile_pool(name="gb", bufs=2) as gpb,
            tc.tile_pool(name="hst", bufs=16) as hpool,
            tc.tile_pool(name="ps", bufs=8, space="PSUM") as pspool,
            tc.tile_pool(name="fin", bufs=2) as fpool,
        ):
            idx_t = consts.tile([128, idx_cols], i16)
            nc.sync.dma_start(out=idx_t[:], in_=idx_in[:])
            wh_t = consts.tile([128, wh_cols], bf16)
            nc.sync.dma_start(out=wh_t[:], in_=wh_in[:])
            bias_t = consts.tile([128, bias_ncol], f32)
            nc.sync.dma_start(out=bias_t[:], in_=bias_in[:])

            # zero the gather pool buffers once so pad lanes stay finite
            prezt = []
            for pool, nb, nm in ((gpa, nblkA, "gA"), (gpb, nblkB, "gB")):
                for _ in range(2):
                    z = pool.tile([128, nb, HB], bf16, name=nm)
                    nc.vector.memset(z[:], 0)
                    prezt.append(z)

            writes = {}   # (h-layer 1|2, half) -> list of dma insts
            hstages = {}  # (layer, half, tile) -> sbuf tile

            def emit_gathers(li, lay, half, gtile, classes):
                if mode == "compute_only":
                    return
                for (c, maxc, b0, b1) in lay["segs"]:
                    if c not in classes:
                        continue
                    q = (0 if c == 0 else 2) + half
                    cnt = (b1 - b0) * 128
                    c0 = idx_off[li] + b0 * 8
                    src = xt_in if c == 0 else accT
                    gi = nc.gpsimd.dma_gather(
                        out_ap=gtile[:, b0:b1, :],
                        in_ap=src[:, half * HB:(half + 1) * HB],
                        idxs_ap=idx_t[:, c0:c0 + cnt // 16],
                        num_idxs=cnt,
                        num_idxs_reg=cnt,
                        elem_size=HB,
                        elem_step=BS,
                        single_packet=False,
                        queue_num=q,
                    )
                    for hc in range(1, maxc + 1):
                        for w in writes.get((hc, half), []):
                            tile.add_dep_helper(gi.ins, w.ins, sync=True)

            def emit_compute(li, lay, half, gtile, hidden):
                o = lay["o"]
                wl = []
                if mode == "gather_only":
                    return wl
                for ch in range(NCH):
                    ps_tiles = []
                    for t in range(lay["ntile"]):
                        ps = pspool.tile([128, 512], f32, name="ps")
                        ps_tiles.append(ps)
                    prev_inst = {}
                    for mm in lay["mms"]:
                        mi = nc.tensor.matmul(
                            out=ps_tiles[mm.tile][mm.c0:mm.c1, :],
                            lhsT=wh_t[:, mm.w_off:mm.w_off + mm.w],
                            rhs=gtile[:, mm.blk, ch * 512:(ch + 1) * 512],
                            start=mm.start,
                            stop=mm.stop,
                            skip_group_check=True,
                            tile_position=(0, mm.c0),
                        )
                        if mm.tile in prev_inst:
                            tile.add_dep_helper(
                                mi.ins, prev_inst[mm.tile].ins, sync=False)
                        prev_inst[mm.tile] = mi
                    for si, (st, w_off, w) in enumerate(lay["scat_mms"]):
                        h3 = hstages[(2, half, st)]
                        mi = nc.tensor.matmul(
                            out=ps_tiles[0][0:w, :],
                            lhsT=wh_t[:, w_off:w_off + w],
                            rhs=h3[:, ch * 512:(ch + 1) * 512],
                            start=False,
                            stop=si == len(lay["scat_mms"]) - 1,
                            skip_group_check=True,
                            tile_position=(0, 0),
                        )
                        if 0 in prev_inst:
                            tile.add_dep_helper(
                                mi.ins, prev_inst[0].ins, sync=False)
                        prev_inst[0] = mi
                    for t in range(lay["ntile"]):
                        m = min(lay["gpt"], (G if hidden else GO_G) - t * lay["gpt"]) * o
                        bcol = lay["bt0"] + t
                        if hidden:
                            if ch == 0:
                                hstages[(li, half, t)] = hpool.tile(
                                    [128, HB], bf16, name="hs")
                            hs = hstages[(li, half, t)]
                            nc.scalar.activation(
                                out=hs[0:m, ch * 512:(ch + 1) * 512],
                                in_=ps_tiles[t][0:m, :],
                                func=mybir.ActivationFunctionType.Tanh,
                                bias=bias_t[0:m, bcol:bcol + 1],
                                scale=1.0,
                            )
                            if ch == NCH - 1 and li < 2:
                                r0 = li * G * O + t * 128
                                wr = nc.sync.dma_start(
                                    out=accT[r0:r0 + m,
                                             half * HB:(half + 1) * HB],
                                    in_=hs[0:m, :])
                                wl.append(wr)
                        else:
                            nc.vector.tensor_scalar_add(
                                out=yT[0:m, ch * 512:(ch + 1) * 512],
                                in0=ps_tiles[t][0:m, :],
                                scalar1=bias_t[0:m, bcol:bcol + 1],
                            )
                return wl

            def body(iv=None):
                global yT
                writes.clear()
                gt = {}
                if mode == "compute_only":
                    for h in (0, 1):
                        gt[(0, h)] = gt[(2, h)] = prezt[h]
                        gt[(1, h)] = gt[(3, h)] = prezt[2 + h]
                    for h in (0, 1):
                        writes[(1, h)] = emit_compute(0, L[0], h, gt[(0, h)], True)
                    for h in (0, 1):
                        writes[(2, h)] = emit_compute(1, L[1], h, gt[(1, h)], True)
                    for h in (0, 1):
                        emit_compute(2, L[2], h, gt[(2, h)], True)
                    for h in (0, 1):
                        yT = fpool.tile([GO_G * GO_O, HB], f32, name="yT")
                        emit_compute(3, lo, h, gt[(3, h)], False)
                        nc.sync.dma_start(out=y_out[h], in_=yT[:])
                    return
                for h in (0, 1):
                    gt[(0, h)] = gpa.tile([128, nblkA, HB], bf16, name="gA")
                    emit_gathers(0, L[0], h, gt[(0, h)], {0})
                for h in (0, 1):
                    gt[(1, h)] = gpb.tile([128, nblkB, HB], bf16, name="gB")
                    emit_gathers(1, L[1], h, gt[(1, h)], {0})
                for h in (0, 1):
                    writes[(1, h)] = emit_compute(0, L[0], h, gt[(0, h)], True)
                    gt[(2, h)] = gpa.tile([128, nblkA, HB], bf16, name="gA")
                    emit_gathers(1, L[1], h, gt[(1, h)], {1})
                    emit_gathers(2, L[2], h, gt[(2, h)], {0, 1})
                for h in (0, 1):
                    writes[(2, h)] = emit_compute(1, L[1], h, gt[(1, h)], True)
                    gt[(3, h)] = gpb.tile([128, nblkB, HB], bf16, name="gB")
                    emit_gathers(2, L[2], h, gt[(2, h)], {2})
                    emit_gathers(3, lo, h, gt[(3, h)], {0, 1})
                for h in (0, 1):
                    emit_compute(2, L[2], h, gt[(2, h)], True)
                for h in (0, 1):
                    if mode == "gather_only":
                        continue
                    yT = fpool.tile([GO_G * GO_O, HB], f32, name="yT")
                    emit_compute(3, lo, h, gt[(3, h)], False)
                    nc.sync.dma_start(out=y_out[h], in_=yT[:])

            if with_loop:
                nit_t = consts.tile([1, 1], mybir.dt.int32)
                nc.sync.dma_start(out=nit_t[:], in_=nit_in[:])
                n = nc.values_load(nit_t[0:1, 0:1], min_val=0, max_val=2048,
                                   skip_runtime_bounds_check=True)
                with tc.For_i(0, n, 1):
                    for _ in range(UNROLL):
                        body()
            else:
                for _ in range(nbody):
                    body()

    nc.compile()
    return nc


class _Runner:
    """Persistent jitted SPMD executable (adapted from bass2jax)."""

    def __init__(self, nc):
        import jax
        import concourse.mybir as mybir
        from jax.sharding import Mesh, PartitionSpec
        from jax.experimental.shard_map import shard_map
        from concourse.bass2jax import (
            _bass_exec_p, partition_id_tensor, install_neuronx_cc_hook,
        )

        install_neuronx_cc_hook()
        self.jax = jax
        in_names, out_names, out_avals, zero_outs = [], [], [], []
        partition_name = (
            nc.partition_id_tensor.name if nc.partition_id_tensor else None
        )
        for alloc in nc.m.functions[0].allocations:
            if not isinstance(alloc, mybir.MemoryLocationSet):
                continue
            name = alloc.memorylocations[0].name
            if alloc.kind == "ExternalInput":
                if name != partition_name:
                    in_names.append(name)
            elif alloc.kind == "ExternalOutput":
                out_names.append(name)
                shape = tuple(alloc.tensor_shape)
                dtype = mybir.dt.np(alloc.dtype)
                out_avals.append(jax.core.ShapedArray(shape, dtype))
                zero_outs.append(np.zeros(shape, dtype))
        self.n_params = len(in_names)
        self.in_names = in_names[:]
        self.out_names = out_names
        self.out_avals = out_avals
        self.zero_outs = zero_outs
        all_in = in_names + out_names + ([partition_name] if partition_name else [])
        donate = tuple(range(self.n_params, self.n_params + len(out_names)))

        def _body(*args):
            operands = list(args)
            if partition_name is not None:
                operands.append(partition_id_tensor())
            return tuple(
                _bass_exec_p.bind(
                    *operands,
                    out_avals=tuple(out_avals),
                    in_names=tuple(all_in),
                    out_names=tuple(out_names),
                    lowering_input_output_aliases=(),
                    sim_require_finite=True,
                    sim_require_nnan=True,
                    nc=nc,
                )
            )

        devices = jax.devices()[:N_CORES]
        self.mesh = Mesh(np.asarray(devices), ("core",))
        self.sharded = jax.jit(
            shard_map(
                _body, mesh=self.mesh,
                in_specs=(PartitionSpec("core"),) * (self.n_params + len(out_names)),
                out_specs=(PartitionSpec("core"),) * len(out_names),
                check_rep=False,
            ),
            donate_argnums=donate,
            keep_unused=True,
        )

    def prep(self, in_maps, device_put=True):
        per_core = [[np.asarray(m[name]) for name in self.in_names] for m in in_maps]
        arrs = [
            np.concatenate([per_core[c][i] for c in range(N_CORES)], axis=0)
            for i in range(self.n_params)
        ]
        if device_put:
            from jax.sharding import NamedSharding, PartitionSpec

            sh = NamedSharding(self.mesh, PartitionSpec("core"))
            arrs = [self.jax.device_put(a, sh) for a in arrs]
            self.jax.block_until_ready(arrs)
        return arrs

    def run(self, concat_in):
        zeros = [
            np.zeros((N_CORES * z.shape[0], *z.shape[1:]), z.dtype)
            for z in self.zero_outs
        ]
        outs = self.sharded(*concat_in, *zeros)
        self.jax.block_until_ready(outs)
        return outs

    def split(self, out_arrs):
        return [
            {
                name: np.asarray(out_arrs[i]).reshape(
                    N_CORES, *self.out_avals[i].shape
                )[c]
                for i, name in enumerate(self.out_names)
            }
            for c in range(N_CORES)
        ]


def _get(plan_key, plan, with_loop, mode="full"):
    key = (plan_key, with_loop, mode)
    if key not in _cache:
        nc = _build_program(plan, with_loop, mode=mode)
        _cache[key] = _Runner(nc)
    return _cache[key]


def _in_maps(plan, x, niter):
    bf = ml_dtypes.bfloat16
    x = np.asarray(x)
    maps = []
    for c in range(N_CORES):
        xs = x[c * BS:(c + 1) * BS, :]
        m = {
            "xt": np.ascontiguousarray(xs.T).astype(bf),
            "idxw": plan["idx_wrapped"],
            "wh": plan["wh"],
            "biasp": plan["bias"],
        }
        if niter is not None:
            m["niter"] = np.array([[niter]], np.int32)
        maps.append(m)
    return maps


def _post(y_dev):
    """[2, 64, HB] device output -> [BS, 64]."""
    return np.concatenate([y_dev[0].T, y_dev[1].T], axis=0)


def kernel(**inputs):
    niter = inputs.pop("_niter", None)
    x = inputs.pop("x")
    plan = build_plan(**{k: inputs[k] for k in (
        "idx1", "idx2", "idx3", "idxo", "W1", "W2", "W3", "Wo",
        "b1", "b2", "b3", "bo")})
    r = _get("p0", plan, niter is not None)
    ci = r.prep(_in_maps(plan, x, niter), device_put=False)
    outs = r.split(r.run(ci))
    return np.concatenate(
        [_post(outs[c]["y"]) for c in range(N_CORES)], axis=0
    ).astype(np.float32)


def bench(inputs, k_hi=33, rounds=8, per=4, mode="full"):
    """On-device time per model evaluation: each loop trip runs UNROLL
    evaluations; median over interleaved A/B rounds of
    (wall(k_hi) - wall(1)) / ((k_hi - 1) * UNROLL)."""
    import time

    inputs = dict(inputs)
    x = inputs.pop("x")
    plan = build_plan(**{k: inputs[k] for k in (
        "idx1", "idx2", "idx3", "idxo", "W1", "W2", "W3", "Wo",
        "b1", "b2", "b3", "bo")})
    r = _get("p0", plan, True, mode)
    ci1 = r.prep(_in_maps(plan, x, 1), device_put=True)
    cih = r.prep(_in_maps(plan, x, k_hi), device_put=True)
    outs = r.split(r.run(ci1))
    y1 = np.concatenate(
        [_post(outs[c]["y"]) for c in range(N_CORES)], axis=0).astype(np.float32)
    outs = r.split(r.run(cih))
    yh = np.concatenate(
        [_post(outs[c]["y"]) for c in range(N_CORES)], axis=0).astype(np.float32)
    diffs = []
    for _ in range(rounds):
        t1s, ths = [], []
        for _ in range(per):
            t0 = time.perf_counter(); r.run(ci1)
            t1s.append(time.perf_counter() - t0)
            t0 = time.perf_counter(); r.run(cih)
            ths.append(time.perf_counter() - t0)
        diffs.append((min(ths) - min(t1s)) / ((k_hi - 1) * UNROLL))
    diffs.sort()
    return diffs[len(diffs) // 2], y1, yh


# revision 28
# speedup vs baseline: 1.7146x; 1.2261x over previous
"""Trainium2 Bass kernel for nn_DirectEncodingModel (gnn_message_passing).
Dependency-class pipelined gather design, data-parallel over 8 cores.

v3 changes vs baseline:
  - Batch split in 2 halves (1024 cols each); gather pools double-buffered
    so halves and loop iterations pipeline (fills the DMA idle gaps during
    L2/L3 compute and overlaps next-iter L1 gathers with this-iter tail).
  - Output layer's h3-sourced slots computed via merged "scatter-weight"
    matmuls directly from the SBUF-resident h3 tiles: h3 is never written
    to HBM and never gathered back (removes the full-h3-write -> tiny-
    gather serial tail).
  - accT shrinks to h1+h2 (2048 rows); class index mapping v-512.
  - Class pad slots use idx=-1 (trailing negatives move no bytes); gather
    pool buffers are memset once pre-loop so pad lanes stay finite.
  - y emitted transposed [2, 64, 1024] and untransposed on host (drops the
    PE transposes + strided y DMA).
"""

import numpy as np
import ml_dtypes

B = 16384
IN = 512
G, F, O = 64, 24, 16
GO_G, GO_F, GO_O = 4, 48, 16
N_CORES = 8
BS = B // N_CORES          # 2048 batch per core
HB = BS // 2               # 1024 per half
NCH = HB // 512            # 2 chunks per half
HROWS = 2 * G * O          # 2048 rows in accT (h1+h2)
H3_BASE = IN + 2 * G * O   # 2560: global col where h3 starts
UNROLL = 4                 # computes per hardware loop iteration

_cache = {}


def _cls_of(v):
    return 0 if v < IN else 1 + (v - IN) // (G * O)


class MM:
    __slots__ = ("blk", "tile", "c0", "c1", "start", "w_off", "w", "stop")

    def __init__(self, blk, tile, c0, c1, start, w_off, w):
        self.blk, self.tile, self.c0, self.c1 = blk, tile, c0, c1
        self.start, self.w_off, self.w = start, w_off, w
        self.stop = False


def build_plan(idx1, idx2, idx3, idxo, W1, W2, W3, Wo, b1, b2, b3, bo):
    """Packed stationary weights + per-layer block/mm/gather structure."""
    wcols = []          # stationary column groups [128, w]
    bias_cols = []      # [128] per (layer, ptile)
    layers = []

    def pack_layer(idx, W, b, n_g, fan, o, scatter_cls=(), merge_h=False):
        gpt = 128 // o  # groups per psum tile
        slots = []
        scat = []
        for g in range(n_g):
            for f in range(fan):
                v = int(idx[g, f])
                c = _cls_of(v)
                if c in scatter_cls:
                    scat.append((g, v, W[g, f, :]))
                else:
                    slots.append((c, g, v, W[g, f, :]))
        # optionally merge all h classes into one segment (same source
        # tensor accT); the gather waits for the latest h-layer referenced.
        ckey = (lambda c: min(c, 1)) if merge_h else (lambda c: c)
        slots.sort(key=lambda s: (ckey(s[0]), s[1]))
        segs_raw = {}
        for s in slots:
            segs_raw.setdefault(ckey(s[0]), []).append(s)
        padded = []
        segs = []  # (key 0|1, maxc, b0, b1)
        for c in sorted(segs_raw):
            lst = segs_raw[c]
            maxc = max(s[0] for s in lst)
            b0 = len(padded) // 128
            npad = (-len(lst)) % 128
            lst = lst + [(c, None, -1, np.zeros(o))] * npad
            padded.extend(lst)
            b1_ = len(padded) // 128
            segs.append((c, maxc, b0, b1_))
        nblk = len(padded) // 128
        idx_arr = np.zeros(nblk * 128, np.int16)
        for i, s in enumerate(padded):
            v = s[2]
            idx_arr[i] = -1 if s[1] is None else (v if s[0] == 0 else v - IN)
        used_w = min(gpt, n_g) * o
        mms = []
        tile_first_pending = set(range((n_g + gpt - 1) // gpt))
        for blk in range(nblk):
            bslots = padded[blk * 128:(blk + 1) * 128]
            groups = sorted({s[1] for s in bslots if s[1] is not None})
            if not groups:
                continue
            tiles = sorted({g // gpt for g in groups})
            for t in tiles:
                tg = [g for g in groups if g // gpt == t]
                ga, gb = tg[0], tg[-1]
                first = t in tile_first_pending
                if first:
                    tile_first_pending.discard(t)
                    c0, c1 = 0, used_w
                else:
                    c0 = ((ga % gpt) * o) // 32 * 32
                    c1 = -((-((gb % gpt + 1) * o)) // 32) * 32
                    c1 = min(c1, used_w)
                pieces = []
                p0 = c0
                while p0 < c1:
                    for bw in (128, 64, 32):
                        if p0 % bw == 0 and p0 + bw <= ((c1 + 31) // 32) * 32:
                            pieces.append((p0, min(p0 + bw, c1)))
                            p0 = p0 + bw
                            break
                for (p0c, p1c) in pieces:
                    w = p1c - p0c
                    stat = np.zeros((128, w), np.float32)
                    for r, s in enumerate(bslots):
                        if s[1] is not None and s[1] // gpt == t:
                            cc = (s[1] % gpt) * o - p0c
                            if 0 <= cc <= w - o:
                                stat[r, cc:cc + o] = s[3]
                    w_off = sum(wc.shape[1] for wc in wcols)
                    wcols.append(stat)
                    mms.append(MM(blk, t, p0c, p1c, first, w_off, w))
        # scatter-weight matmuls: one per 128-row h3 source tile that is
        # actually referenced; stationary [128 h3-rows, used_w] merges all
        # scattered slots hitting that tile.
        scat_mms = []  # (src_tile, w_off, w)
        if scat:
            by_tile = {}
            for (g, v, wrow) in scat:
                r = v - H3_BASE
                by_tile.setdefault(r // 128, []).append((r % 128, g, wrow))
            for st in sorted(by_tile):
                stat = np.zeros((128, used_w), np.float32)
                for (p, g, wrow) in by_tile[st]:
                    stat[p, g * o:(g + 1) * o] += wrow
                w_off = sum(wc.shape[1] for wc in wcols)
                wcols.append(stat)
                scat_mms.append((st, w_off, used_w))
        if mms:
            last = {}
            for i, m in enumerate(mms):
                last[m.tile] = i
            for t, i in last.items():
                if not (t == 0 and scat_mms):
                    mms[i].stop = True
        ntile = (n_g + gpt - 1) // gpt
        bt0 = len(bias_cols)
        for t in range(ntile):
            col = np.zeros(128, np.float32)
            for gi in range(min(gpt, n_g - t * gpt)):
                col[gi * o:(gi + 1) * o] = b[t * gpt + gi, :]
            bias_cols.append(col)
        return dict(nblk=nblk, idx=idx_arr, segs=segs, mms=mms,
                    scat_mms=scat_mms, ntile=ntile, gpt=gpt, o=o, bt0=bt0)

    for idx, W, b in ((idx1, W1, b1), (idx2, W2, b2), (idx3, W3, b3)):
        layers.append(pack_layer(np.asarray(idx), np.asarray(W),
                                 np.asarray(b), G, F, O))
    lo = pack_layer(np.asarray(idxo), np.asarray(Wo), np.asarray(bo),
                    GO_G, GO_F, GO_O, scatter_cls=(3,), merge_h=True)
    wh = np.concatenate(wcols, axis=1)
    bias = np.stack(bias_cols, axis=1)  # [128, ncols]
    idx_all = np.concatenate([layers[0]["idx"], layers[1]["idx"],
                              layers[2]["idx"], lo["idx"]])
    return dict(layers=layers, lo=lo, wh=wh.astype(ml_dtypes.bfloat16),
                bias=bias.astype(np.float32),
                idx_wrapped=_wrap(idx_all), idx_total=idx_all.shape[0])


def _wrap(idx_list):
    n = idx_list.shape[0]
    w = idx_list.reshape(n // 16, 16).T
    return np.tile(w, (8, 1)).astype(np.int16)


def _apply_tile_patch():
    from concourse import tile as _tile
    from concourse.vector_clock import ScopedClock, VectorClock

    def _patched(self, tick_clock, wait_clock):
        nc = self.nc
        vc = tick_clock.global_clock
        for proc in range(len(vc)):
            tick = vc[proc]
            if tick > 0:
                nop_inst = nc.sync.nop(hint="drain_split_wait", nofuse=True)
                single = VectorClock()
                single.require_at_least(proc, tick)
                wait_clock.add_sem_waits(nop_inst.ins, ScopedClock({None: single}))
        nc.sync.drain()
        nc.all_engine_barrier()
        assert self.sems is not None
        popped = nc._tile_sem_poison_stack.pop()
        assert popped is self._sem_poison
        nc.clear_and_free_semaphores(list(self.sems.allocated().values()))
        nc.all_engine_barrier()

    _tile.TileContext._drain_and_barrier = _patched


def _build_program(plan, with_loop, nbody=1, mode="full"):
    from concourse import bacc
    import concourse.mybir as mybir
    import concourse.tile as tile

    _apply_tile_patch()
    f32, bf16, i16 = mybir.dt.float32, mybir.dt.bfloat16, mybir.dt.int16
    L = plan["layers"]
    lo = plan["lo"]
    wh_cols = plan["wh"].shape[1]
    bias_ncol = plan["bias"].shape[1]
    idx_cols = plan["idx_total"] // 16

    nc = bacc.Bacc(
        "TRN2", target_bir_lowering=False, debug=False, num_devices=N_CORES,
        enable_asserts=False, num_swdge_queues=4,
        dynamic_dma_scratch_size=32768,
    )
    xt_in = nc.dram_tensor("xt", [IN, BS], bf16, kind="ExternalInput")
    idx_in = nc.dram_tensor("idxw", [128, idx_cols], i16, kind="ExternalInput")
    wh_in = nc.dram_tensor("wh", [128, wh_cols], bf16, kind="ExternalInput")
    bias_in = nc.dram_tensor("biasp", [128, bias_ncol], f32, kind="ExternalInput")
    y_out = nc.dram_tensor("y", [2, GO_G * GO_O, HB], f32, kind="ExternalOutput")
    accT = nc.dram_tensor("accT", [HROWS, BS], bf16)
    if with_loop:
        nit_in = nc.dram_tensor("niter", [1, 1], mybir.dt.int32,
                                kind="ExternalInput")

    idx_off = [0]
    for lay in L:
        idx_off.append(idx_off[-1] + lay["nblk"] * 8)
    # queues: per-half lanes. q0/q1 = x-class H0/H1, q2/q3 = h-class H0/H1.
    nblkA = max(L[0]["nblk"], L[2]["nblk"])
    nblkB = max(L[1]["nblk"], lo["nblk"])

    with tile.TileContext(nc) as tc:
        with (
            tc.tile_pool(name="const", bufs=1) as consts,
            tc.tile_pool(name="ga", bufs=2) as gpa,
            tc.tile_pool(name="gb", bufs=2) as gpb,
            tc.tile_pool(name="hst", bufs=16) as hpool,
            tc.tile_pool(name="ps", bufs=8, space="PSUM") as pspool,
            tc.tile_pool(name="fin", bufs=2) as fpool,
        ):
            idx_t = consts.tile([128, idx_cols], i16)
            nc.sync.dma_start(out=idx_t[:], in_=idx_in[:])
            wh_t = consts.tile([128, wh_cols], bf16)
            nc.sync.dma_start(out=wh_t[:], in_=wh_in[:])
            bias_t = consts.tile([128, bias_ncol], f32)
            nc.sync.dma_start(out=bias_t[:], in_=bias_in[:])

            # zero the gather pool buffers once so pad lanes stay finite
            prezt = []
            for pool, nb, nm in ((gpa, nblkA, "gA"), (gpb, nblkB, "gB")):
                for _ in range(2):
                    z = pool.tile([128, nb, HB], bf16, name=nm)
                    nc.vector.memset(z[:], 0)
                    prezt.append(z)

            writes = {}   # (h-layer 1|2, half) -> list of dma insts
            hstages = {}  # (layer, half, tile) -> sbuf tile

            def emit_gathers(li, lay, half, gtile, classes):
                if mode == "compute_only":
                    return
                for (c, maxc, b0, b1) in lay["segs"]:
                    if c not in classes:
                        continue
                    src = xt_in if c == 0 else accT
                    # h-class gathers sit on the inter-layer critical path:
                    # split them across two queues so their transfer halves.
                    if c == 0 or b1 - b0 < 2:
                        splits = [(b0, b1, (0 if c == 0 else 2) + half)]
                    else:
                        bm = (b0 + b1 + 1) // 2
                        splits = [(b0, bm, 2 + half), (bm, b1, 0 + half)]
                    for (sb, eb, q) in splits:
                        cnt = (eb - sb) * 128
                        c0 = idx_off[li] + sb * 8
                        gi = nc.gpsimd.dma_gather(
                            out_ap=gtile[:, sb:eb, :],
                            in_ap=src[:, half * HB:(half + 1) * HB],
                            idxs_ap=idx_t[:, c0:c0 + cnt // 16],
                            num_idxs=cnt,
                            num_idxs_reg=cnt,
                            elem_size=HB,
                            elem_step=BS,
                            single_packet=False,
                            queue_num=q,
                        )
                        for hc in range(1, maxc + 1):
                            for w in writes.get((hc, half), []):
                                tile.add_dep_helper(gi.ins, w.ins, sync=True)

            def emit_compute(li, lay, half, gtile, hidden):
                o = lay["o"]
                wl = []
                if mode == "gather_only":
                    return wl
                if not hidden:
                    for ch in range(NCH):
                        ps = pspool.tile([128, 512], f32, name="ps", bufs=2)
                        prev = None
                        for mm in lay["mms"]:
                            mi = nc.tensor.matmul(
                                out=ps[mm.c0:mm.c1, :],
                                lhsT=wh_t[:, mm.w_off:mm.w_off + mm.w],
                                rhs=gtile[:, mm.blk, ch * 512:(ch + 1) * 512],
                                start=mm.start,
                                stop=mm.stop,
                                skip_group_check=True,
                                tile_position=(0, mm.c0),
                            )
                            if prev is not None:
                                tile.add_dep_helper(mi.ins, prev.ins, sync=False)
                            prev = mi
                        for si, (st, w_off, w) in enumerate(lay["scat_mms"]):
                            h3 = hstages[(2, half, st)]
                            mi = nc.tensor.matmul(
                                out=ps[0:w, :],
                                lhsT=wh_t[:, w_off:w_off + w],
                                rhs=h3[:, ch * 512:(ch + 1) * 512],
                                start=False,
                                stop=si == len(lay["scat_mms"]) - 1,
                                skip_group_check=True,
                                tile_position=(0, 0),
                            )
                            if prev is not None:
                                tile.add_dep_helper(mi.ins, prev.ins, sync=False)
                            prev = mi
                        m = GO_G * o
                        nc.vector.tensor_scalar_add(
                            out=yT[0:m, ch * 512:(ch + 1) * 512],
                            in0=ps[0:m, :],
                            scalar1=bias_t[0:m, lay["bt0"]:lay["bt0"] + 1],
                        )
                    return wl
                # hidden layer: 2-bank psum per tile, tiles in 2 groups of 4;
                # one full-width ACT per tile (fewer ACT dispatches, earlier
                # h writes for the downstream gathers).
                for tgrp in range(0, lay["ntile"], 3):
                    tset = range(tgrp, min(tgrp + 3, lay["ntile"]))
                    ps_tiles = {t: pspool.tile([128, NCH * 512], f32,
                                               name="ps2", bufs=3)
                                for t in tset}
                    prev_inst = {}
                    for ch in range(NCH):
                        for mm in lay["mms"]:
                            if mm.tile not in ps_tiles:
                                continue
                            mi = nc.tensor.matmul(
                                out=ps_tiles[mm.tile][mm.c0:mm.c1,
                                                      ch * 512:(ch + 1) * 512],
                                lhsT=wh_t[:, mm.w_off:mm.w_off + mm.w],
                                rhs=gtile[:, mm.blk, ch * 512:(ch + 1) * 512],
                                start=mm.start,
                                stop=mm.stop,
                                skip_group_check=True,
                                tile_position=(0, mm.c0),
                            )
                            if mm.tile in prev_inst:
                                tile.add_dep_helper(
                                    mi.ins, prev_inst[mm.tile].ins, sync=False)
                            prev_inst[mm.tile] = mi
                    for t in tset:
                        m = min(lay["gpt"], G - t * lay["gpt"]) * o
                        bcol = lay["bt0"] + t
                        hstages[(li, half, t)] = hpool.tile(
                            [128, HB], bf16, name="hs")
                        hs = hstages[(li, half, t)]
                        nc.scalar.activation(
                            out=hs[0:m, :],
                            in_=ps_tiles[t][0:m, :],
                            func=mybir.ActivationFunctionType.Tanh,
                            bias=bias_t[0:m, bcol:bcol + 1],
                            scale=1.0,
                        )
                        if li < 2:
                            r0 = li * G * O + t * 128
                            wr = nc.sync.dma_start(
                                out=accT[r0:r0 + m,
                                         half * HB:(half + 1) * HB],
                                in_=hs[0:m, :])
                            wl.append(wr)
                return wl

            def body(iv=None):
                global yT
                writes.clear()
                gt = {}
                if mode == "compute_only":
                    for h in (0, 1):
                        gt[(0, h)] = gt[(2, h)] = prezt[h]
                        gt[(1, h)] = gt[(3, h)] = prezt[2 + h]
                    for h in (0, 1):
                        writes[(1, h)] = emit_compute(0, L[0], h, gt[(0, h)], True)
                    for h in (0, 1):
                        writes[(2, h)] = emit_compute(1, L[1], h, gt[(1, h)], True)
                    for h in (0, 1):
                        emit_compute(2, L[2], h, gt[(2, h)], True)
                    for h in (0, 1):
                        yT = fpool.tile([GO_G * GO_O, HB], f32, name="yT")
                        emit_compute(3, lo, h, gt[(3, h)], False)
                        nc.sync.dma_start(out=y_out[h], in_=yT[:])
                    return
                for h in (0, 1):
                    gt[(0, h)] = gpa.tile([128, nblkA, HB], bf16, name="gA")
                    emit_gathers(0, L[0], h, gt[(0, h)], {0})
                for h in (0, 1):
                    gt[(1, h)] = gpb.tile([128, nblkB, HB], bf16, name="gB")
                    emit_gathers(1, L[1], h, gt[(1, h)], {0})
                for h in (0, 1):
                    writes[(1, h)] = emit_compute(0, L[0], h, gt[(0, h)], True)
                    gt[(2, h)] = gpa.tile([128, nblkA, HB], bf16, name="gA")
                    emit_gathers(1, L[1], h, gt[(1, h)], {1})
                    emit_gathers(2, L[2], h, gt[(2, h)], {0, 1})
                for h in (0, 1):
                    writes[(2, h)] = emit_compute(1, L[1], h, gt[(1, h)], True)
                    gt[(3, h)] = gpb.tile([128, nblkB, HB], bf16, name="gB")
                    emit_gathers(2, L[2], h, gt[(2, h)], {2})
                    emit_gathers(3, lo, h, gt[(3, h)], {0, 1})
                for h in (0, 1):
                    emit_compute(2, L[2], h, gt[(2, h)], True)
                for h in (0, 1):
                    if mode == "gather_only":
                        continue
                    yT = fpool.tile([GO_G * GO_O, HB], f32, name="yT")
                    emit_compute(3, lo, h, gt[(3, h)], False)
                    nc.sync.dma_start(out=y_out[h], in_=yT[:])

            if with_loop:
                nit_t = consts.tile([1, 1], mybir.dt.int32)
                nc.sync.dma_start(out=nit_t[:], in_=nit_in[:])
                n = nc.values_load(nit_t[0:1, 0:1], min_val=0, max_val=2048,
                                   skip_runtime_bounds_check=True)
                with tc.For_i(0, n, 1):
                    for _ in range(UNROLL):
                        body()
            else:
                for _ in range(nbody):
                    body()

    nc.compile()
    return nc


class _Runner:
    """Persistent jitted SPMD executable (adapted from bass2jax)."""

    def __init__(self, nc):
        import jax
        import concourse.mybir as mybir
        from jax.sharding import Mesh, PartitionSpec
        from jax.experimental.shard_map import shard_map
        from concourse.bass2jax import (
            _bass_exec_p, partition_id_tensor, install_neuronx_cc_hook,
        )

        install_neuronx_cc_hook()
        self.jax = jax
        in_names, out_names, out_avals, zero_outs = [], [], [], []
        partition_name = (
            nc.partition_id_tensor.name if nc.partition_id_tensor else None
        )
        for alloc in nc.m.functions[0].allocations:
            if not isinstance(alloc, mybir.MemoryLocationSet):
                continue
            name = alloc.memorylocations[0].name
            if alloc.kind == "ExternalInput":
                if name != partition_name:
                    in_names.append(name)
            elif alloc.kind == "ExternalOutput":
                out_names.append(name)
                shape = tuple(alloc.tensor_shape)
                dtype = mybir.dt.np(alloc.dtype)
                out_avals.append(jax.core.ShapedArray(shape, dtype))
                zero_outs.append(np.zeros(shape, dtype))
        self.n_params = len(in_names)
        self.in_names = in_names[:]
        self.out_names = out_names
        self.out_avals = out_avals
        self.zero_outs = zero_outs
        all_in = in_names + out_names + ([partition_name] if partition_name else [])
        donate = tuple(range(self.n_params, self.n_params + len(out_names)))

        def _body(*args):
            operands = list(args)
            if partition_name is not None:
                operands.append(partition_id_tensor())
            return tuple(
                _bass_exec_p.bind(
                    *operands,
                    out_avals=tuple(out_avals),
                    in_names=tuple(all_in),
                    out_names=tuple(out_names),
                    lowering_input_output_aliases=(),
                    sim_require_finite=True,
                    sim_require_nnan=True,
                    nc=nc,
                )
            )

        devices = jax.devices()[:N_CORES]
        self.mesh = Mesh(np.asarray(devices), ("core",))
        self.sharded = jax.jit(
            shard_map(
                _body, mesh=self.mesh,
                in_specs=(PartitionSpec("core"),) * (self.n_params + len(out_names)),
                out_specs=(PartitionSpec("core"),) * len(out_names),
                check_rep=False,
            ),
            donate_argnums=donate,
            keep_unused=True,
        )

    def prep(self, in_maps, device_put=True):
        per_core = [[np.asarray(m[name]) for name in self.in_names] for m in in_maps]
        arrs = [
            np.concatenate([per_core[c][i] for c in range(N_CORES)], axis=0)
            for i in range(self.n_params)
        ]
        if device_put:
            from jax.sharding import NamedSharding, PartitionSpec

            sh = NamedSharding(self.mesh, PartitionSpec("core"))
            arrs = [self.jax.device_put(a, sh) for a in arrs]
            self.jax.block_until_ready(arrs)
        return arrs

    def run(self, concat_in):
        zeros = [
            np.zeros((N_CORES * z.shape[0], *z.shape[1:]), z.dtype)
            for z in self.zero_outs
        ]
        outs = self.sharded(*concat_in, *zeros)
        self.jax.block_until_ready(outs)
        return outs

    def split(self, out_arrs):
        return [
            {
                name: np.asarray(out_arrs[i]).reshape(
                    N_CORES, *self.out_avals[i].shape
                )[c]
                for i, name in enumerate(self.out_names)
            }
            for c in range(N_CORES)
        ]


def _get(plan_key, plan, with_loop, mode="full"):
    key = (plan_key, with_loop, mode)
    if key not in _cache:
        nc = _build_program(plan, with_loop, mode=mode)
        _cache[key] = _Runner(nc)
    return _cache[key]


def _in_maps(plan, x, niter):
    bf = ml_dtypes.bfloat16
    x = np.asarray(x)
    maps = []
    for c in range(N_CORES):
        xs = x[c * BS:(c + 1) * BS, :]
        m = {
            "xt": np.ascontiguousarray(xs.T).astype(bf),
            "idxw": plan["idx_wrapped"],
            "wh": plan["wh"],
            "biasp": plan["bias"],
        }
        if niter is not None:
            m["niter"] = np.array([[niter]], np.int32)
        maps.append(m)
    return maps


def _post(y_dev):
    """[2, 64, HB] device output -> [BS, 64]."""
    return np.concatenate([y_dev[0].T, y_dev[1].T], axis=0)


def kernel(**inputs):
    niter = inputs.pop("_niter", None)
    x = inputs.pop("x")
    plan = build_plan(**{k: inputs[k] for k in (
        "idx1", "idx2", "idx3", "idxo", "W1", "W2", "W3", "Wo",
        "b1", "b2", "b3", "bo")})
    r = _get("p0", plan, niter is not None)
    ci = r.prep(_in_maps(plan, x, niter), device_put=False)
    outs = r.split(r.run(ci))
    return np.concatenate(
        [_post(outs[c]["y"]) for c in range(N_CORES)], axis=0
    ).astype(np.float32)


def bench(inputs, k_hi=33, rounds=8, per=4, mode="full"):
    """On-device time per model evaluation: each loop trip runs UNROLL
    evaluations; median over interleaved A/B rounds of
    (wall(k_hi) - wall(1)) / ((k_hi - 1) * UNROLL)."""
    import time

    inputs = dict(inputs)
    x = inputs.pop("x")
    plan = build_plan(**{k: inputs[k] for k in (
        "idx1", "idx2", "idx3", "idxo", "W1", "W2", "W3", "Wo",
        "b1", "b2", "b3", "bo")})
    r = _get("p0", plan, True, mode)
    ci1 = r.prep(_in_maps(plan, x, 1), device_put=True)
    cih = r.prep(_in_maps(plan, x, k_hi), device_put=True)
    outs = r.split(r.run(ci1))
    y1 = np.concatenate(
        [_post(outs[c]["y"]) for c in range(N_CORES)], axis=0).astype(np.float32)
    outs = r.split(r.run(cih))
    yh = np.concatenate(
        [_post(outs[c]["y"]) for c in range(N_CORES)], axis=0).astype(np.float32)
    diffs = []
    for _ in range(rounds):
        t1s, ths = [], []
        for _ in range(per):
            t0 = time.perf_counter(); r.run(ci1)
            t1s.append(time.perf_counter() - t0)
            t0 = time.perf_counter(); r.run(cih)
            ths.append(time.perf_counter() - t0)
        diffs.append((min(ths) - min(t1s)) / ((k_hi - 1) * UNROLL))
    diffs.sort()
    return diffs[len(diffs) // 2], y1, yh
